# revision 5
# baseline (speedup 1.0000x reference)
"""Channel-wise tensor product (e3nn-style) Trainium2 Bass kernel.

out[n] = concat(o0, o1, o2, o3, o4) with
  o0[u]      = w0[u] * s0[u] * y0
  o1[u,k]    = w1[u] * s0[u] * y1[k]
  o2[u,i]    = w2[u] * s1[u,i] * y0
  o3[u]      = w3[u]/sqrt(3) * sum_i s1[u,i] y1[i]
  o4[u,k]    = w4[u]/sqrt(2) * (s1[u,:] x y1)[k]

Sharding: pure data parallel over the batch dim across 8 cores.
Layout: batch rows on SBUF partitions (128-row tiles), channels on the
free dim. Per-row scalars y0/y1 live in per-partition scalar operands;
per-channel weights are replicated across partitions host-side with the
CG normalization folded in.
"""

import numpy as np

import concourse.bass as bass
import concourse.tile as tile
from concourse import bacc, mybir
from concourse.bass_utils import run_bass_kernel_spmd

N_CORES = 8
B = 65536
U = 128
ROWS = B // N_CORES          # 8192 rows per core
NT = ROWS // 128             # 64 tiles of 128 rows
SQRT2 = 1.4142135623730951
SQRT3 = 1.7320508075688772

F32 = mybir.dt.float32
MUL = mybir.AluOpType.mult
ADD = mybir.AluOpType.add
SUB = mybir.AluOpType.subtract
COPY = mybir.ActivationFunctionType.Copy


def build_nc() -> bass.Bass:
    nc = bacc.Bacc("TRN2", target_bir_lowering=False, debug=False)

    x1s = nc.dram_tensor("x1s", (ROWS, 4 * U), F32, kind="ExternalInput").ap()
    x2s = nc.dram_tensor("x2s", (128, 4 * NT), F32, kind="ExternalInput").ap()
    wbig = nc.dram_tensor("wbig", (128, 11 * U), F32, kind="ExternalInput").ap()
    out = nc.dram_tensor("out", (ROWS, 11 * U), F32, kind="ExternalOutput").ap()

    with tile.TileContext(nc) as tc:
        with (
            tc.tile_pool(name="const", bufs=1) as cpool,
            tc.tile_pool(name="xin", bufs=4) as xpool,
            tc.tile_pool(name="prod", bufs=3) as ppool,
            tc.tile_pool(name="outp", bufs=4) as opool,
            tc.tile_pool(name="tmp", bufs=3) as epool,
        ):
            WB = cpool.tile([128, 11 * U], F32)
            nc.sync.dma_start(WB[:], wbig[:])
            X2 = cpool.tile([128, 4 * NT], F32)
            nc.sync.dma_start(X2[:], x2s[:])

            for t in range(NT):
                X = xpool.tile([128, 4 * U], F32)
                nc.sync.dma_start(X[:], x1s[t * 128:(t + 1) * 128, :])

                y0 = X2[:, 4 * t:4 * t + 1]

                # P_j = X * y1_j  (j = 0..2), each (128, 512): the s0 part
                # is path-1's a_j, the s1 part holds path-3/4 products.
                P = ppool.tile([128, 3 * 4 * U], F32)
                for j in range(3):
                    yj = X2[:, 4 * t + 1 + j:4 * t + 2 + j]
                    nc.scalar.activation(
                        P[:, j * 512:(j + 1) * 512], X[:], COPY, scale=yj
                    )

                O = opool.tile([128, 11 * U], F32)

                # path 0: o0 = (s0 * y0) * w0
                nc.vector.scalar_tensor_tensor(
                    O[:, 0:U], X[:, 0:U], y0, WB[:, 0:U], MUL, MUL
                )
                # path 2: o2 = (s1 * y0) * w2   (interleaved u,i layout)
                nc.vector.scalar_tensor_tensor(
                    O[:, 4 * U:7 * U], X[:, U:4 * U], y0, WB[:, 4 * U:7 * U],
                    MUL, MUL,
                )
                # path 1: o1[u,k] = a_k[u] * w1[u]; dest col 128+3u+k.
                o1dst = O[:, U:4 * U].rearrange("p (u k) -> p k u", k=3)
                a_src = P[:].rearrange("p (j s) -> p j s", j=3)[:, :, 0:U]
                w1v = WB[:, U:4 * U].rearrange("p (u k) -> p k u", k=3)
                nc.vector.tensor_tensor(o1dst, a_src, w1v, MUL)

                # path 3 on GPSIMD: o3 = (d0 + d1 + d2) * w3'
                # d_j = P_j[s1 part, diag i=j] = P[:, 512j+128+j :: 3]
                E = epool.tile([128, U], F32)
                d0 = P[:, 512 * 0 + U + 0:512 * 0 + 4 * U:3]
                d1 = P[:, 512 * 1 + U + 1:512 * 1 + 4 * U:3]
                d2 = P[:, 512 * 2 + U + 2:512 * 2 + 4 * U:3]
                nc.gpsimd.tensor_tensor(E[:], d0, d1, ADD)
                nc.gpsimd.tensor_tensor(E[:], E[:], d2, ADD)
                nc.gpsimd.tensor_tensor(
                    O[:, 7 * U:8 * U], E[:], WB[:, 7 * U:8 * U], MUL
                )

                # path 4: o4[u,k] = (c[k+1,k+2] - c[k+2,k+1]) * w4'
                # c[i,j] = P_j[s1 part, component i] = P[:, 512j+128+i :: 3]
                for k in range(3):
                    i1, j1 = (k + 1) % 3, (k + 2) % 3
                    i2, j2 = (k + 2) % 3, (k + 1) % 3
                    a = P[:, 512 * j1 + U + i1:512 * j1 + 4 * U:3]
                    b = P[:, 512 * j2 + U + i2:512 * j2 + 4 * U:3]
                    o4k = O[:, 8 * U + k:11 * U:3]
                    nc.vector.tensor_tensor(o4k, a, b, SUB)
                nc.vector.tensor_tensor(
                    O[:, 8 * U:11 * U], O[:, 8 * U:11 * U],
                    WB[:, 8 * U:11 * U], MUL,
                )

                nc.sync.dma_start(out[t * 128:(t + 1) * 128, :], O[:])

    nc.compile()
    return nc


def _host_prep(x1, x2, weight):
    """Shard x1/x2 per core; build the replicated weight layout."""
    x1 = np.ascontiguousarray(x1, dtype=np.float32)
    x2 = np.ascontiguousarray(x2, dtype=np.float32)
    w = np.asarray(weight, dtype=np.float32).reshape(5, U)

    wrow = np.concatenate([
        w[0],
        np.repeat(w[1], 3),
        np.repeat(w[2], 3),
        w[3] / SQRT3,
        np.repeat(w[4], 3) / SQRT2,
    ])
    wbig = np.ascontiguousarray(np.broadcast_to(wrow, (128, 11 * U)))

    in_maps = []
    for c in range(N_CORES):
        x1c = x1[c * ROWS:(c + 1) * ROWS]
        x2c = x2[c * ROWS:(c + 1) * ROWS]
        # x2s[p, 4t+c] = x2c[t*128+p, c]
        x2c = np.ascontiguousarray(
            x2c.reshape(NT, 128, 4).transpose(1, 0, 2).reshape(128, 4 * NT)
        )
        in_maps.append({"x1s": x1c, "x2s": x2c, "wbig": wbig})
    return in_maps


_NC_CACHE = {}


def _ensure_ntff_hook():
    """The agent image lacks antenv.axon_hooks; synthesize it so
    run_bass_kernel_spmd(trace=True) can register the NTFF profiler."""
    import sys
    import types

    try:
        import antenv.axon_hooks  # noqa: F401
        return
    except ImportError:
        pass
    mod = types.ModuleType("antenv.axon_hooks")
    state = {"hook": None}

    def set_axon_ntff_profile_hook(hook):
        state["hook"] = hook

    def get_axon_ntff_profile_hook():
        if state["hook"] is None:
            import os

            so = "/opt/axon/libaxon_pjrt.so"
            if os.path.exists(so):
                try:
                    from trn_agent_boot.trn_boot import _ntff_profile_via_ctypes

                    state["hook"] = _ntff_profile_via_ctypes(so)
                except Exception:
                    state["hook"] = None
        return state["hook"]

    mod.set_axon_ntff_profile_hook = set_axon_ntff_profile_hook
    mod.get_axon_ntff_profile_hook = get_axon_ntff_profile_hook
    sys.modules["antenv.axon_hooks"] = mod


def kernel(x1, x2, weight, trace=False):
    assert x1.shape == (B, 4 * U) and x2.shape == (B, 4)
    if trace:
        _ensure_ntff_hook()
    in_maps = _host_prep(x1, x2, weight)
    if "nc" not in _NC_CACHE:
        _NC_CACHE["nc"] = build_nc()
    nc = _NC_CACHE["nc"]
    res = run_bass_kernel_spmd(
        nc, in_maps, core_ids=list(range(N_CORES)), trace=trace
    )
    out = np.concatenate([res.results[c]["out"] for c in range(N_CORES)], axis=0)
    if trace:
        kernel.last_exec_time_ns = res.exec_time_ns
        kernel.last_results = res
    return out
